# revision 9
# baseline (speedup 1.0000x reference)
"""CTC loss on 8 trn2 NeuronCores.

Design:
- Batch B=64 split 8/core for the memory-bound part: each core streams its
  own 27MB of predicts through ACT exp(+accum) for the log_softmax
  denominators, which factor out of the CTC DP entirely
  (loss = -(ln L + renorms - sum_t ln denom_t)).
- The T=128-step CTC DP runs in linear space with periodic renorm. The
  serial chain is split in half across core pairs: even cores run the
  FORWARD chain for the pair's 16 samples, odd cores the BACKWARD
  (suffix) chain, both as the *identical* SPMD program — the direction
  lives entirely in host-prepared data (s-axis reversed for backward,
  transition masks baked in as -1e30 logits, E_127 absorbed into the
  backward init). Both chains are 63 steps of 3 fused DVE ops + 1 final
  multiply; cores exchange chain states with a pairwise AllGather and
  combine L = sum_s alpha_63[s] * gamma_63[s].
"""

from contextlib import ExitStack

import numpy as np

import concourse.bacc as bacc
import concourse.tile as tile
import concourse.mybir as mybir
from concourse.ap import AP
from concourse.bass_utils import run_bass_kernel_spmd

B, T, C, L = 64, 128, 6625, 25
S = 2 * L + 1  # 51
M = 8          # cores
BS = B // M    # own samples per core (denominator stream)
PS = 2 * BS    # pair samples per core (DP chain)
NSTEP = 63
RENORM = 8
NREN = 8       # 7 in-chain renorms + 1 post-final
CHUNKS = [(0, 3313), (3313, 3312)]
F32 = mybir.dt.float32

_cached = {}


def _dup_free(ap, n):
    """AP reading the free range of `ap` n times: [.., (0,n), (step,cnt)]."""
    dims = [list(d) for d in ap.ap]
    return AP(ap.tensor, ap.offset, dims[:-1] + [[0, n]] + [dims[-1]])


def _rev_free(ap):
    """AP reading the innermost free dim of `ap` reversed."""
    dims = [list(d) for d in ap.ap]
    st, ct = dims[-1]
    return AP(ap.tensor, ap.offset + st * (ct - 1), dims[:-1] + [[-st, ct]])


def _strided2(ap, gap, n):
    """AP over `ap`'s tensor writing two n-wide blocks `gap` apart."""
    dims = [list(d) for d in ap.ap]
    return AP(ap.tensor, ap.offset, dims[:-1] + [[gap, 2], [1, n]])


def _build():
    if "nc" in _cached:
        return _cached["nc"]
    nc = bacc.Bacc(
        "TRN2", target_bir_lowering=False, debug=False, num_devices=M
    )
    x = nc.dram_tensor("x", [BS, T, C], F32, kind="ExternalInput").ap()
    gcat = nc.dram_tensor("gcat", [PS, NSTEP * 2 * S], F32,
                          kind="ExternalInput").ap()
    gfin = nc.dram_tensor("gfin", [PS, S], F32, kind="ExternalInput").ap()
    yinit = nc.dram_tensor("yinit", [PS, S], F32, kind="ExternalInput").ap()
    ownsel = nc.dram_tensor("ownsel", [PS, BS], F32, kind="ExternalInput").ap()
    loss = nc.dram_tensor("loss", [BS, 1], F32, kind="ExternalOutput").ap()

    EXP = mybir.ActivationFunctionType.Exp
    LN = mybir.ActivationFunctionType.Ln

    with tile.TileContext(nc) as tc, ExitStack() as ctx:
        cpool = ctx.enter_context(tc.tile_pool(name="consts", bufs=1))
        xpool = ctx.enter_context(tc.tile_pool(name="xs", bufs=3))
        epool = ctx.enter_context(tc.tile_pool(name="es", bufs=2))
        pspool = ctx.enter_context(tc.tile_pool(name="ps", bufs=1, space="PSUM"))
        dram = ctx.enter_context(tc.tile_pool(name="dram", bufs=1, space="DRAM"))

        # --- small inputs ---
        g_sb = cpool.tile([PS, NSTEP * 2 * S], F32)
        gfin_sb = cpool.tile([PS, S], F32)
        y_sb = cpool.tile([PS, S], F32)
        osel_sb = cpool.tile([PS, BS], F32)
        nc.sync.dma_start(g_sb[:], gcat)
        nc.sync.dma_start(gfin_sb[:], gfin)
        nc.sync.dma_start(y_sb[:], yinit)
        nc.sync.dma_start(osel_sb[:], ownsel)

        # --- bulk exp of chain factors (first ACT ops: unblock the DP) ---
        e_sb = cpool.tile([PS, NSTEP * 2 * S], F32)
        nc.scalar.activation(e_sb[:], g_sb[:], EXP)
        efin_sb = cpool.tile([PS, S], F32)
        nc.scalar.activation(efin_sb[:], gfin_sb[:], EXP)

        # --- DP chain: 63 steps of 3 fused DVE ops ---
        # wcat layout: [pad2 | w(51) | pad2 | wc(51)] = 106 cols
        wcat = cpool.tile([PS, 2 * S + 4], F32)
        u_t = cpool.tile([PS, S], F32)
        ys = cpool.tile([PS, NREN], F32)
        inv = cpool.tile([PS, 1], F32)
        nc.vector.memset(wcat[:], 0.0)

        # blocks at cols 2..52 and 55..105 -> gap = 53
        w_view = _strided2(wcat[:, 2 : 2 + S], 53, S)
        jren = 0
        for k in range(1, NSTEP + 1):
            ek = e_sb[:, (k - 1) * 2 * S : k * 2 * S].rearrange(
                "p (two s) -> p two s", two=2
            )
            nc.vector.tensor_mul(w_view, _dup_free(y_sb[:], 2), ek)
            nc.vector.tensor_add(u_t[:], wcat[:, 2 : 2 + S], wcat[:, 1 : 1 + S])
            nc.vector.tensor_add(y_sb[:], u_t[:], wcat[:, S + 2 : 2 * S + 2])
            if k % RENORM == 0:
                nc.vector.reduce_max(ys[:, jren : jren + 1], y_sb[:],
                                     axis=mybir.AxisListType.X)
                nc.vector.reciprocal(inv[:], ys[:, jren : jren + 1])
                nc.vector.tensor_scalar_mul(y_sb[:], y_sb[:], inv[:])
                jren += 1

        # final multiply (fwd: E_63; bwd: ones), then one more renorm
        xfin = cpool.tile([PS, S], F32)
        nc.vector.tensor_mul(xfin[:], y_sb[:], efin_sb[:])
        nc.vector.reduce_max(ys[:, jren : jren + 1], xfin[:],
                             axis=mybir.AxisListType.X)
        nc.vector.reciprocal(inv[:], ys[:, jren : jren + 1])
        nc.vector.tensor_scalar_mul(xfin[:], xfin[:], inv[:])
        jren += 1
        assert jren == NREN

        # --- pairwise exchange: [X(51) | ys(8)] -> AllGather over pairs ---
        xpack = cpool.tile([PS, S + NREN], F32)
        nc.vector.tensor_copy(xpack[:, 0:S], xfin[:])
        nc.vector.tensor_copy(xpack[:, S : S + NREN], ys[:])
        ib = dram.tile([PS, S + NREN], F32, tag="ib")
        ob = dram.tile([2 * PS, S + NREN], F32, tag="ob")
        nc.sync.dma_start(ib[:], xpack[:])
        nc.gpsimd.collective_compute(
            "AllGather", mybir.AluOpType.bypass,
            replica_groups=[[0, 1], [2, 3], [4, 5], [6, 7]],
            ins=[ib.opt()], outs=[ob.opt()],
        )
        gbe = cpool.tile([PS, S + NREN], F32)
        gbo = cpool.tile([PS, S + NREN], F32)
        nc.sync.dma_start(gbe[:], ob[0:PS, :])
        nc.sync.dma_start(gbo[:], ob[PS : 2 * PS, :])

        # --- denominator stream (the memory-bound part) ---
        denp = cpool.tile([128, 2 * BS], F32)
        den_all = cpool.tile([128, BS], F32)
        for b in range(BS):
            for ci, (c0, cw) in enumerate(CHUNKS):
                xt = xpool.tile([128, cw], F32, tag="xt")
                nc.sync.dma_start(xt[:], x[b, :, c0 : c0 + cw])
                et2 = epool.tile([128, cw], F32, tag="et2")
                idx = 2 * b + ci
                nc.scalar.activation(
                    et2[:], xt[:], EXP, accum_out=denp[:, idx : idx + 1]
                )
            nc.vector.tensor_add(
                den_all[:, b : b + 1], denp[:, 2 * b : 2 * b + 1],
                denp[:, 2 * b + 1 : 2 * b + 2],
            )
        ld_all = cpool.tile([128, BS], F32)
        nc.scalar.activation(ld_all[:], den_all[:], LN)
        ones = cpool.tile([128, 1], F32)
        nc.vector.memset(ones[:], 1.0)

        # --- combine: L = sum_s X_f[s] * X_b[50-s] per pair sample ---
        prod = cpool.tile([PS, S], F32)
        nc.vector.tensor_mul(prod[:], gbe[:, 0:S], _rev_free(gbo[:, 0:S]))
        lv = cpool.tile([PS, 1], F32)
        nc.vector.reduce_sum(lv[:], prod[:], axis=mybir.AxisListType.X)
        lnl = cpool.tile([PS, 1], F32)
        nc.scalar.activation(lnl[:], lv[:], LN)
        lyse = cpool.tile([PS, NREN], F32)
        lyso = cpool.tile([PS, NREN], F32)
        nc.scalar.activation(lyse[:], gbe[:, S : S + NREN], LN)
        nc.scalar.activation(lyso[:], gbo[:, S : S + NREN], LN)
        lacce = cpool.tile([PS, 1], F32)
        lacco = cpool.tile([PS, 1], F32)
        nc.vector.reduce_sum(lacce[:], lyse[:], axis=mybir.AxisListType.X)
        nc.vector.reduce_sum(lacco[:], lyso[:], axis=mybir.AxisListType.X)
        tot_a = cpool.tile([PS, 1], F32)
        tot = cpool.tile([PS, 1], F32)
        nc.vector.tensor_add(tot_a[:], lnl[:], lacce[:])
        nc.vector.tensor_add(tot[:], tot_a[:], lacco[:])
        ntot = cpool.tile([PS, 1], F32)
        nc.vector.tensor_scalar_mul(ntot[:], tot[:], -1.0)

        # loss = sum_t ln(denom) - (lnL + lacc_f + lacc_b), via PSUM accumulate
        lsum = pspool.tile([BS, 1], F32)
        nc.tensor.matmul(lsum[:], lhsT=ld_all[:], rhs=ones[:],
                         start=True, stop=False)
        nc.tensor.matmul(lsum[:], lhsT=osel_sb[:], rhs=ntot[:],
                         start=False, stop=True)
        loss_sb = cpool.tile([BS, 1], F32)
        nc.vector.tensor_copy(loss_sb[:], lsum[:])
        nc.sync.dma_start(loss, loss_sb[:])

    nc.compile()
    _cached["nc"] = nc
    return nc


def _host_prep(predicts, labels, label_lengths):
    predicts = np.ascontiguousarray(np.asarray(predicts, dtype=np.float32))
    labels = np.asarray(labels).astype(np.int64)
    lens = np.asarray(label_lengths).astype(np.int64)

    ext = np.zeros((B, S), np.int64)
    ext[:, 1::2] = labels
    ext_sm2 = np.zeros((B, S), np.int64)
    ext_sm2[:, 2:] = ext[:, :-2]
    skip = ((ext != 0) & (ext != ext_sm2)).astype(np.float32)  # m[s]

    g = np.take_along_axis(predicts, ext[:, None, :], axis=2)  # [B,T,S] f32
    se = (2 * lens).astype(np.int64)
    for b in range(B):
        g[b, :, se[b] + 1 :] = -1e30  # s>2*len never feeds back

    endm = np.zeros((B, S), np.float32)
    endm[np.arange(B), se] = 1.0
    endm[np.arange(B), se - 1] = 1.0

    NEG = np.float32(-1e30)
    in_maps = []
    for m in range(M):
        p = m // 2
        sl = slice(16 * p, 16 * p + PS)       # pair samples
        gp, skp, enp = g[sl], skip[sl], endm[sl]
        gc = np.full((PS, NSTEP, 2, S), NEG, np.float32)
        gf = np.zeros((PS, S), np.float32)
        yi = np.zeros((PS, S), np.float32)
        if m % 2 == 0:
            # forward: step k consumes E_{k-1}; A=g[k-1,s]; C=g[k-1,s'] if m[s'+2]
            for k in range(1, NSTEP + 1):
                gc[:, k - 1, 0, :] = gp[:, k - 1, :]
                cm = np.full((PS, S), NEG, np.float32)
                cm[:, : S - 2] = np.where(skp[:, 2:] > 0, gp[:, k - 1, : S - 2], NEG)
                gc[:, k - 1, 1, :] = cm
            gf[:] = gp[:, NSTEP, :]           # E_63
            yi[:, 0] = 1.0
            yi[:, 1] = 1.0
        else:
            # backward, s-reversed; step k consumes E_{127-k} (k=1..63)
            gr = gp[:, :, ::-1]               # \hat g
            mr = skp[:, ::-1]                 # \hat m
            for k in range(1, NSTEP + 1):
                t = T - 2 - k                 # 125 .. 63; consumes E_{t+1}
                gc[:, k - 1, 0, :] = gr[:, t + 1, :]
                gc[:, k - 1, 1, :] = np.where(mr > 0, gr[:, t + 1, :], NEG)
            # init absorbs E_127: gamma_126 then reverse
            w = np.exp(gp[:, T - 1, :]) * enp
            wm = skp * w
            gm = w.copy()
            gm[:, : S - 1] += w[:, 1:]
            gm[:, : S - 2] += wm[:, 2:]
            yi[:] = gm[:, ::-1]
            # gfin stays 0 -> exp = 1
        osel = np.zeros((PS, BS), np.float32)
        off = 0 if m % 2 == 0 else BS
        for b in range(BS):
            osel[off + b, b] = 1.0
        in_maps.append({
            "x": np.ascontiguousarray(predicts[m * BS : (m + 1) * BS]),
            "gcat": np.ascontiguousarray(gc.reshape(PS, NSTEP * 2 * S)),
            "gfin": gf,
            "yinit": yi,
            "ownsel": osel,
        })
    return in_maps


def _run(in_maps, trace=False):
    nc = _build()
    res = run_bass_kernel_spmd(nc, in_maps, list(range(M)), trace=trace)
    losses = np.concatenate(
        [res.results[m]["loss"].reshape(BS) for m in range(M)]
    )
    losses = np.where(losses < 1e29, losses, 0.0).astype(np.float32)
    out = np.asarray(losses.mean(), dtype=np.float32)
    return out, res


def kernel(predicts, labels, label_lengths):
    in_maps = _host_prep(predicts, labels, label_lengths)
    out, _ = _run(in_maps, trace=False)
    return out


def kernel_traced(predicts, labels, label_lengths):
    in_maps = _host_prep(predicts, labels, label_lengths)
    return _run(in_maps, trace=True)


# revision 13
# speedup vs baseline: 1.1526x; 1.1526x over previous
"""CTC loss on 8 trn2 NeuronCores.

Design:
- Batch B=64 split 8/core for the memory-bound part: each core streams its
  own 27MB of predicts through ACT exp(+accum) for the log_softmax
  denominators, which factor out of the CTC DP entirely
  (loss = -(ln L + renorms - sum_t ln denom_t)).
- The T=128-step CTC DP runs in linear space with periodic renorm. The
  serial chain is split in half across core pairs: even cores run the
  FORWARD chain for the pair's 16 samples, odd cores the BACKWARD
  (suffix) chain, both as the *identical* SPMD program — the direction
  lives entirely in host-prepared data (s-axis reversed for backward,
  transition masks baked in as -1e30 logits, E_127 absorbed into the
  backward init). Both chains are 63 steps of 3 fused DVE ops + 1 final
  multiply; cores exchange chain states with a pairwise AllGather and
  combine L = sum_s alpha_63[s] * gamma_63[s].
"""

from contextlib import ExitStack

import numpy as np

import concourse.bacc as bacc
import concourse.tile as tile
import concourse.mybir as mybir
from concourse.ap import AP
from concourse.bass_utils import run_bass_kernel_spmd

B, T, C, L = 64, 128, 6625, 25
S = 2 * L + 1  # 51
M = 8          # cores
BS = B // M    # own samples per core (denominator stream)
PS = 2 * BS    # pair samples per core (DP chain)
NSTEP = 63
NSLOT = 64     # 63 steps + final-multiply slot
RENORM = 8
NREN = 8       # 7 in-chain renorms + 1 pre-final
ECH = 16       # chain slots per exp chunk
CHUNKS = [(0, 3313), (3313, 3312)]
F32 = mybir.dt.float32

_cached = {}


def _dup_free(ap, n):
    """AP reading the free range of `ap` n times: [.., (0,n), (step,cnt)]."""
    dims = [list(d) for d in ap.ap]
    return AP(ap.tensor, ap.offset, dims[:-1] + [[0, n]] + [dims[-1]])


def _rev_free(ap):
    """AP reading the innermost free dim of `ap` reversed."""
    dims = [list(d) for d in ap.ap]
    st, ct = dims[-1]
    return AP(ap.tensor, ap.offset + st * (ct - 1), dims[:-1] + [[-st, ct]])


def _strided2(ap, gap, n):
    """AP over `ap`'s tensor writing two n-wide blocks `gap` apart."""
    dims = [list(d) for d in ap.ap]
    return AP(ap.tensor, ap.offset, dims[:-1] + [[gap, 2], [1, n]])


def _build():
    if "nc" in _cached:
        return _cached["nc"]
    nc = bacc.Bacc(
        "TRN2", target_bir_lowering=False, debug=False, num_devices=M
    )
    x = nc.dram_tensor("x", [BS, T, C], F32, kind="ExternalInput").ap()
    gcat = nc.dram_tensor("gcat", [PS, NSLOT * 2 * S], F32,
                          kind="ExternalInput").ap()
    yinit = nc.dram_tensor("yinit", [PS, S], F32, kind="ExternalInput").ap()
    ownsel = nc.dram_tensor("ownsel", [PS, BS], F32, kind="ExternalInput").ap()
    loss = nc.dram_tensor("loss", [BS, 1], F32, kind="ExternalOutput").ap()

    EXP = mybir.ActivationFunctionType.Exp
    LN = mybir.ActivationFunctionType.Ln
    MULT = mybir.AluOpType.mult
    CW = 2 * S * ECH  # exp chunk width

    with tile.TileContext(nc) as tc, ExitStack() as ctx:
        cpool = ctx.enter_context(tc.tile_pool(name="consts", bufs=1))
        xpool = ctx.enter_context(tc.tile_pool(name="xs", bufs=3))
        epool = ctx.enter_context(tc.tile_pool(name="es", bufs=2))
        pspool = ctx.enter_context(tc.tile_pool(name="ps", bufs=1, space="PSUM"))
        dram = ctx.enter_context(tc.tile_pool(name="dram", bufs=1, space="DRAM"))

        # --- small inputs (gpsimd SWDGE: off the stream's sync queue) ---
        gts = [cpool.tile([PS, CW], F32, tag=f"gt{i}", name=f"gt{i}") for i in range(4)]
        for i in range(4):
            nc.gpsimd.dma_start(gts[i][:], gcat[:, i * CW : (i + 1) * CW])
        y_sb = cpool.tile([PS, S], F32)
        osel_sb = cpool.tile([PS, BS], F32)
        nc.gpsimd.dma_start(y_sb[:], yinit)
        nc.gpsimd.dma_start(osel_sb[:], ownsel)

        # --- chunked exp of chain factors (DP starts after chunk 0) ---
        ets = [cpool.tile([PS, CW], F32, tag=f"et{i}", name=f"et{i}") for i in range(4)]
        for i in range(4):
            nc.scalar.activation(ets[i][:], gts[i][:], EXP)

        # --- DP chain: 63 steps of 3 fused DVE ops ---
        # wcat layout: [pad2 | w(51) | pad2 | wc(51)] = 106 cols
        wcat = cpool.tile([PS, 2 * S + 4], F32)
        u_t = cpool.tile([PS, S], F32)
        xpack = cpool.tile([PS, S + NREN], F32)  # [X(51) | ys(8)]
        inv = cpool.tile([PS, 1], F32)
        nc.vector.memset(wcat[:], 0.0)

        w_view = _strided2(wcat[:, 2 : 2 + S], 53, S)
        ys = xpack[:, S : S + NREN]
        jren = 0
        pending = False
        for k in range(1, NSTEP + 1):
            ci, off = (k - 1) // ECH, ((k - 1) % ECH) * 2 * S
            ek = ets[ci][:, off : off + 2 * S].rearrange(
                "p (two s) -> p two s", two=2
            )
            if pending:
                nc.vector.scalar_tensor_tensor(
                    w_view, _dup_free(y_sb[:], 2), inv[:], ek, MULT, MULT
                )
                pending = False
            else:
                nc.vector.tensor_mul(w_view, _dup_free(y_sb[:], 2), ek)
            nc.vector.tensor_add(u_t[:], wcat[:, 2 : 2 + S], wcat[:, 1 : 1 + S])
            nc.vector.tensor_add(y_sb[:], u_t[:], wcat[:, S + 2 : 2 * S + 2])
            if k % RENORM == 0:
                nc.vector.reduce_max(ys[:, jren : jren + 1], y_sb[:],
                                     axis=mybir.AxisListType.X)
                nc.vector.reciprocal(inv[:], ys[:, jren : jren + 1])
                pending = True
                jren += 1

        # final multiply (slot 64 A-half: fwd E_63 / bwd ones) + renorm
        nc.vector.reduce_max(ys[:, jren : jren + 1], y_sb[:],
                             axis=mybir.AxisListType.X)
        nc.vector.reciprocal(inv[:], ys[:, jren : jren + 1])
        jren += 1
        assert jren == NREN
        foff = (NSTEP % ECH) * 2 * S
        efin = ets[3][:, foff : foff + S]
        nc.vector.scalar_tensor_tensor(
            xpack[:, 0:S], y_sb[:], inv[:], efin, MULT, MULT
        )

        # --- pairwise exchange ---
        ib = dram.tile([PS, S + NREN], F32, tag="ib")
        ob = dram.tile([2 * PS, S + NREN], F32, tag="ob")
        nc.sync.dma_start(ib[:], xpack[:])
        nc.gpsimd.collective_compute(
            "AllGather", mybir.AluOpType.bypass,
            replica_groups=[[0, 1], [2, 3], [4, 5], [6, 7]],
            ins=[ib.opt()], outs=[ob.opt()],
        )
        gbe = cpool.tile([PS, S + NREN], F32)
        gbo = cpool.tile([PS, S + NREN], F32)
        nc.sync.dma_start(gbe[:], ob[0:PS, :])
        nc.sync.dma_start(gbo[:], ob[PS : 2 * PS, :])

        # --- combine: L = sum_s X_f[s] * X_b[50-s] per pair sample ---
        prod = cpool.tile([PS, S], F32)
        nc.vector.tensor_mul(prod[:], gbe[:, 0:S], _rev_free(gbo[:, 0:S]))
        lv = cpool.tile([PS, 1], F32)
        nc.vector.reduce_sum(lv[:], prod[:], axis=mybir.AxisListType.X)
        lnl = cpool.tile([PS, 1], F32)
        nc.scalar.activation(lnl[:], lv[:], LN)
        lyse = cpool.tile([PS, NREN], F32)
        lyso = cpool.tile([PS, NREN], F32)
        nc.scalar.activation(lyse[:], gbe[:, S : S + NREN], LN)
        nc.scalar.activation(lyso[:], gbo[:, S : S + NREN], LN)
        lacce = cpool.tile([PS, 1], F32)
        lacco = cpool.tile([PS, 1], F32)
        nc.vector.reduce_sum(lacce[:], lyse[:], axis=mybir.AxisListType.X)
        nc.vector.reduce_sum(lacco[:], lyso[:], axis=mybir.AxisListType.X)
        tot_a = cpool.tile([PS, 1], F32)
        ntot = cpool.tile([PS, 1], F32)
        nc.vector.tensor_add(tot_a[:], lnl[:], lacce[:])
        nc.vector.scalar_tensor_tensor(
            ntot[:], tot_a[:], -1.0, lacco[:],
            MULT, mybir.AluOpType.subtract,
        )

        # loss = sum_t ln(denom) - (lnL + laccs): PSUM-accumulated matmuls
        lsum = pspool.tile([BS, 1], F32)
        nc.tensor.matmul(lsum[:], lhsT=osel_sb[:], rhs=ntot[:],
                         start=True, stop=False)

        # --- denominator stream (the memory-bound part) ---
        denp = cpool.tile([128, 2 * BS], F32)
        den_all = cpool.tile([128, BS], F32)
        ld_all = cpool.tile([128, BS], F32)
        for b in range(BS):
            for ci, (c0, cw) in enumerate(CHUNKS):
                xt = xpool.tile([128, cw], F32, tag="xt")
                nc.sync.dma_start(xt[:], x[b, :, c0 : c0 + cw])
                et2 = epool.tile([128, cw], F32, tag="et2")
                idx = 2 * b + ci
                nc.scalar.activation(
                    et2[:], xt[:], EXP, accum_out=denp[:, idx : idx + 1]
                )
            nc.vector.tensor_add(
                den_all[:, b : b + 1], denp[:, 2 * b : 2 * b + 1],
                denp[:, 2 * b + 1 : 2 * b + 2],
            )
            nc.scalar.activation(ld_all[:, b : b + 1], den_all[:, b : b + 1], LN)
        ones = cpool.tile([128, 1], F32)
        nc.vector.memset(ones[:], 1.0)
        nc.tensor.matmul(lsum[:], lhsT=ld_all[:], rhs=ones[:],
                         start=False, stop=True)
        loss_sb = cpool.tile([BS, 1], F32)
        nc.vector.tensor_copy(loss_sb[:], lsum[:])
        nc.sync.dma_start(loss, loss_sb[:])

    nc.compile()
    _cached["nc"] = nc
    return nc


def _host_prep(predicts, labels, label_lengths):
    predicts = np.ascontiguousarray(np.asarray(predicts, dtype=np.float32))
    labels = np.asarray(labels).astype(np.int64)
    lens = np.asarray(label_lengths).astype(np.int64)

    ext = np.zeros((B, S), np.int64)
    ext[:, 1::2] = labels
    ext_sm2 = np.zeros((B, S), np.int64)
    ext_sm2[:, 2:] = ext[:, :-2]
    skip = ((ext != 0) & (ext != ext_sm2)).astype(np.float32)  # m[s]

    g = np.take_along_axis(predicts, ext[:, None, :], axis=2)  # [B,T,S] f32
    se = (2 * lens).astype(np.int64)
    for b in range(B):
        g[b, :, se[b] + 1 :] = -1e30  # s>2*len never feeds back

    endm = np.zeros((B, S), np.float32)
    endm[np.arange(B), se] = 1.0
    endm[np.arange(B), se - 1] = 1.0

    NEG = np.float32(-1e30)
    in_maps = []
    for m in range(M):
        p = m // 2
        sl = slice(16 * p, 16 * p + PS)       # pair samples
        gp, skp, enp = g[sl], skip[sl], endm[sl]
        gc = np.full((PS, NSLOT, 2, S), NEG, np.float32)
        yi = np.zeros((PS, S), np.float32)
        if m % 2 == 0:
            # forward: step k consumes E_{k-1}; A=g[k-1,s]; C=g[k-1,s'] if m[s'+2]
            for k in range(1, NSTEP + 1):
                gc[:, k - 1, 0, :] = gp[:, k - 1, :]
                cm = np.full((PS, S), NEG, np.float32)
                cm[:, : S - 2] = np.where(skp[:, 2:] > 0, gp[:, k - 1, : S - 2], NEG)
                gc[:, k - 1, 1, :] = cm
            gc[:, NSTEP, 0, :] = gp[:, NSTEP, :]  # final-mul slot: E_63
            yi[:, 0] = 1.0
            yi[:, 1] = 1.0
        else:
            # backward, s-reversed; init absorbs E_127; steps consume E_126..E_64
            gr = gp[:, :, ::-1]               # \hat g
            mr = skp[:, ::-1]                 # \hat m
            for k in range(1, NSTEP + 1):
                t = T - 2 - k                 # 125 .. 63; consumes E_{t+1}
                gc[:, k - 1, 0, :] = gr[:, t + 1, :]
                gc[:, k - 1, 1, :] = np.where(mr > 0, gr[:, t + 1, :], NEG)
            gc[:, NSTEP, 0, :] = 0.0          # final-mul slot: ones
            w = np.exp(gp[:, T - 1, :]) * enp
            wm = skp * w
            gm = w.copy()
            gm[:, : S - 1] += w[:, 1:]
            gm[:, : S - 2] += wm[:, 2:]
            yi[:] = gm[:, ::-1]
        osel = np.zeros((PS, BS), np.float32)
        off = 0 if m % 2 == 0 else BS
        for b in range(BS):
            osel[off + b, b] = 1.0
        in_maps.append({
            "x": np.ascontiguousarray(predicts[m * BS : (m + 1) * BS]),
            "gcat": np.ascontiguousarray(gc.reshape(PS, NSLOT * 2 * S)),
            "yinit": yi,
            "ownsel": osel,
        })
    return in_maps


def _run(in_maps, trace=False):
    nc = _build()
    res = run_bass_kernel_spmd(nc, in_maps, list(range(M)), trace=trace)
    losses = np.concatenate(
        [res.results[m]["loss"].reshape(BS) for m in range(M)]
    )
    losses = np.where(losses < 1e29, losses, 0.0).astype(np.float32)
    out = np.asarray(losses.mean(), dtype=np.float32)
    return out, res


def kernel(predicts, labels, label_lengths):
    in_maps = _host_prep(predicts, labels, label_lengths)
    out, _ = _run(in_maps, trace=False)
    return out


def kernel_traced(predicts, labels, label_lengths):
    in_maps = _host_prep(predicts, labels, label_lengths)
    return _run(in_maps, trace=True)
